# revision 51
# baseline (speedup 1.0000x reference)
"""DeepseekV2 MLA decode attention (bsz=4, q_len=1, kv_len=2048) on 8 TRN2 cores.

Sharding: tensor-parallel over the 128 heads (16 heads/core).
  - w_q_b / w_kv_b output heads and w_o input heads are sharded.
  - w_q_a is column-sharded (hidden dim) with an on-device AllReduce of the
    tiny [4, 1536] q_a partial (COLLECTIVE=True); KV caches are replicated.
  - Each core computes a partial o_proj output [4, 5120]; the host sums the
    8 partials (the all-reduce of column-parallel o_proj).

Math restructuring (exact up to fp rounding):
  - "Absorbed" MLA: q_eff = W_kv_nope[h].T @ q_nope, scores_nope = q_eff . c
    and o_c = attn @ c, out_v = W_v[h] @ o_c.
  - RoPE folded into host prep (k_pe rotated per-position on host; the
    q-side last-position rotation is folded into w_q_b's pe rows).
  - rmsnorm folded into softmax: scores are computed from the RAW q_a
    (rmsnorm is a per-(batch) positive scale on q, linear through q_b /
    q_eff / scores), and 1/rms enters as a per-partition scale AP on the
    softmax exp. This removes the qan materialization entirely.
  - fp8 (e3m4) streaming for ckv/ckvT/w_o halves their HBM bytes; descales
    are exact powers of two folded into existing constants.
  - Batch-packed attention: partitions 32*b + h hold (batch b, head h) for
    scores / softmax / attn / o_c, so the four batches' small-M matmuls run
    col-tiled (tile_position) in the four 32-col groups of the PE array
    concurrently, and softmax ops process all batches in one [128, 512] op.
  - All DMAs are issued in consumption order up front; the fabric is
    byte-bound (~430 GB/s measured), so the kernel is designed to keep the
    single FIFO DMA stream dense and hide compute under it, with w_o
    (largest, needed last) streamed at the tail.
"""

import numpy as np
from contextlib import ExitStack

HIDDEN = 5120
NUM_HEADS = 128
Q_LORA = 1536
ROPE_D = 64
KV_LORA = 512
V_D = 128
NOPE_D = 128
Q_D = 192
THETA = 10000.0
EPS = 1e-6

N_CORES = 8
HP = NUM_HEADS // N_CORES  # 16 heads per core
BSZ = 4
KV_LEN = 2048

KQ = Q_LORA // 128   # 12
NK5 = KV_LEN // 512  # 4
NK1 = KV_LEN // 128  # 16
ND = KV_LORA // 128  # 4
SCALE = float(Q_D) ** -0.5

COLLECTIVE = True
CC_SINGLETON = False  # timing probe: per-core singleton AllReduce (WRONG output)
STAGE = 'all'  # 'all' | 'dma' (DMA-floor probe: loads only, no compute)

# fp8 (e3m4) streaming for the big HBM tensors. Descale factors are exact
# powers of two folded into existing constants (softmax scale, rmsnorm eps,
# psum-evacuation scales), so the math matches bf16 up to quantization noise.
FP8_CT = True    # ckvT (scores operand): -4.2 MB/core, ~+0.6e-2 rel err
FP8_WO = True    # w_o: -10.5 MB/core, ~+1.1e-2
FP8_C = True     # ckv (o_c operand): -4.2 MB/core, ~+1.0e-2
FP8_WQB = True   # w_q_b: -4.7 MB/core, ~+1.0e-2
S_C = 2.0        # ckv/ckvT/kpe pre-scale (max |ckv| ~5.2 -> 10.4 < 15.5)
S_WO = 128.0     # w_o pre-scale (sigma 0.02 -> 2.56)
S_WQB = 128.0    # w_q_b pre-scale

PW_BUFS = 4    # wqb stream pool
PWO_BUFS = 4   # w_o stream pool
PC_BUFS = 4    # all four batches' c resident (packed o_c)
PCT_BUFS = 4   # all four batches' cT resident (packed scores)
ACC_BUFS = 6
TP_BUFS = 2

_BUILD_CACHE = {}


# --------------------------------------------------------------------------
# host-side prep
# --------------------------------------------------------------------------

def _bf16(x):
    import ml_dtypes
    return np.ascontiguousarray(np.asarray(x, np.float32).astype(ml_dtypes.bfloat16))


def _e3m4(x, scale):
    import ml_dtypes
    xs = np.clip(np.asarray(x, np.float32) * np.float32(scale), -15.5, 15.5)
    return np.ascontiguousarray(xs.astype(ml_dtypes.float8_e3m4))


def _rope_tables():
    exps = np.arange(0, ROPE_D, 2, dtype=np.float32) / np.float32(ROPE_D)
    inv_freq = (np.float32(1.0) / (np.float32(THETA) ** exps)).astype(np.float32)
    ang = np.arange(KV_LEN, dtype=np.float32)[:, None] * inv_freq[None, :]
    cos = np.cos(ang).astype(np.float32)  # [kv, 32]
    sin = np.sin(ang).astype(np.float32)
    return cos, sin


def _swiz(a, p=128):
    """[O*p, N] row-major -> [p, O*N] p-major (row o*p+q lands at [q, o*N:])."""
    o = a.shape[0] // p
    return np.ascontiguousarray(
        a.reshape(o, p, a.shape[1]).transpose(1, 0, 2).reshape(p, o * a.shape[1]))


def _prep_shared(inputs):
    """Host prep shared across cores (replicated tensors)."""
    hidden = np.asarray(inputs["hidden_states"], np.float32)
    ckv = np.asarray(inputs["compressed_kv_normed_cache"], np.float32)
    kpe = np.asarray(inputs["k_pe_cache"], np.float32)
    wqa = np.asarray(inputs["w_q_a"], np.float32)

    hT = np.ascontiguousarray(hidden[:, 0, :].T)           # [5120, 4]
    wqaT = np.ascontiguousarray(wqa.T)                     # [5120, 1536]; sliced per core

    cos, sin = _rope_tables()
    # rotate k_pe on host (per-position rope applied to the cache) and
    # de-interleave: group g={b01,b23}, partition (b%2)*64 + t*32 + f
    kr = kpe[:, :, 0::2]                                   # [4, kv, 32]
    ki = kpe[:, :, 1::2]
    rr = kr * cos[None] - ki * sin[None]
    ri = kr * sin[None] + ki * cos[None]
    k64 = np.concatenate([rr.transpose(0, 2, 1), ri.transpose(0, 2, 1)], axis=1)
    k64 = k64.reshape(2, 128, KV_LEN)
    # with fp8 ckvT, the score psum is S_C*score; scale the kpe operand to
    # match and fold the 1/S_C into the softmax scale constant
    kpeT = _bf16(k64 * S_C) if FP8_CT else _bf16(k64)      # [2, 128, kv]

    # ckv: per batch p-major [128, 16, 512] (kv row o*128+p -> [p, o, :])
    ckv_s = np.stack([_swiz(ckv[b]) for b in range(BSZ)]).reshape(BSZ, 128, NK1, KV_LORA)
    # ckvT: per batch [128 (d%128), 4 (d//128), 2048 kv] — scores operand
    ckvT_s = np.stack([_swiz(np.ascontiguousarray(ckv[b].T)) for b in range(BSZ)])
    ckvT_s = ckvT_s.reshape(BSZ, 128, ND, KV_LEN)

    # batch-broadcast mask: bmask[b, 32b:32b+32] = 1 (psum-partition layout)
    bmask = np.zeros((BSZ, 128), np.float32)
    for b in range(BSZ):
        bmask[b, 32 * b:32 * (b + 1)] = 1.0

    return dict(hT=hT, wqaT=wqaT,
                ckv=_e3m4(ckv_s, S_C) if FP8_C else _bf16(ckv_s),
                ckvT=_e3m4(ckvT_s, S_C) if FP8_CT else _bf16(ckvT_s),
                kpeT=kpeT, bmask=np.ascontiguousarray(bmask),
                cosL=cos[-1], sinL=sin[-1])


def _prep_core(inputs, shared, core):
    wqb = np.asarray(inputs["w_q_b"], np.float32).reshape(NUM_HEADS, Q_D, Q_LORA)
    wkv = np.asarray(inputs["w_kv_b"], np.float32).reshape(NUM_HEADS, NOPE_D + V_D, KV_LORA)
    wo = np.asarray(inputs["w_o"], np.float32)
    ln = np.asarray(inputs["w_q_a_ln"], np.float32)
    cosL, sinL = shared["cosL"], shared["sinL"]            # [32]

    h0 = core * HP
    # per-head q rows: nope, then rotated+de-interleaved pe rows
    # (the last-position rope of q_pe is linear in q -> fold into w_q_b)
    wqb_c = wqb[h0:h0 + HP]                                # [16, 192, 1536]
    wn_rows = wqb_c[:, :NOPE_D, :]                         # [16, 128, 1536]
    wre = wqb_c[:, NOPE_D + 0::2, :]                       # [16, 32, 1536]
    wim = wqb_c[:, NOPE_D + 1::2, :]
    rot_re = cosL[None, :, None] * wre - sinL[None, :, None] * wim
    rot_im = sinL[None, :, None] * wre + cosL[None, :, None] * wim
    # column order: all heads' nope rows first (4 chunks of 4 heads), then
    # all heads' pe rows (2 chunks of 8 heads) — aligns 512-col psum chunks
    # with whole heads for the col-tiled q_b + row-tiled transposes
    pe_rows = np.concatenate([rot_re, rot_im], axis=1)     # [16, 64, 1536]
    wqb_r = np.concatenate([wn_rows.reshape(HP * NOPE_D, Q_LORA),
                            pe_rows.reshape(HP * 64, Q_LORA)], axis=0)
    wqb_r = wqb_r * ln[None, :]                            # [3072, 1536]
    wqbT = _swiz(np.ascontiguousarray(wqb_r.T))            # [128, 12*3072]

    wkv_c = wkv[h0:h0 + HP]                                # [16, 256, 512]
    wnope = _swiz(wkv_c[:, :NOPE_D, :].reshape(HP * NOPE_D, KV_LORA))  # [128,16*512]
    wv = wkv_c[:, NOPE_D:, :]                              # [16, 128, 512]
    wvT = _swiz(np.ascontiguousarray(
        wv.transpose(2, 0, 1).reshape(KV_LORA, HP * V_D)))  # [128, 4*2048]

    woT = np.ascontiguousarray(wo[:, h0 * V_D:(h0 + HP) * V_D].T)  # [2048, 5120]
    # p-major [128, 16, 5120]: partition p holds rows {o*128+p}, contiguous
    woT = np.ascontiguousarray(
        woT.reshape(HP, 128, HIDDEN).transpose(1, 0, 2))
    woT = _e3m4(woT, S_WO) if FP8_WO else _bf16(woT)

    m = dict(ckv=shared["ckv"], ckvT=shared["ckvT"], kpeT=shared["kpeT"],
             bmask=shared["bmask"])
    if COLLECTIVE:
        ksl = HIDDEN // N_CORES
        wqaT = shared["wqaT"][core * ksl:(core + 1) * ksl]
        hT = shared["hT"][core * ksl:(core + 1) * ksl]
    else:
        wqaT, hT = shared["wqaT"], shared["hT"]
    khc = wqaT.shape[0] // 128
    m["wqaT"] = _bf16(_swiz(wqaT)).reshape(128, khc, Q_LORA)
    m["hT"] = _bf16(_swiz(hT)).reshape(128, khc, BSZ)
    wqbT_m = (_e3m4(wqbT, S_WQB) if FP8_WQB else _bf16(wqbT))
    m.update(wqbT=wqbT_m.reshape(128, KQ, HP * Q_D),
             wnope=_bf16(wnope).reshape(128, HP, KV_LORA),
             wvT=_bf16(wvT).reshape(128, ND, HP * V_D),
             woT=woT)
    return m


# --------------------------------------------------------------------------
# device kernel
# --------------------------------------------------------------------------

def _emit_kernel(nc, reps=1, collective=COLLECTIVE):
    import concourse.tile as tile
    import concourse.mybir as mybir
    from concourse.masks import make_identity

    F32 = mybir.dt.float32
    BF = mybir.dt.bfloat16
    F8 = mybir.dt.float8e3
    AX = mybir.AxisListType
    OP = mybir.AluOpType
    ACTF = mybir.ActivationFunctionType

    CT_DT = F8 if FP8_CT else BF
    C_DT = F8 if FP8_C else BF
    WO_DT = F8 if FP8_WO else BF
    WQB_DT = F8 if FP8_WQB else BF
    SC_EFF = SCALE / S_C if FP8_CT else SCALE  # softmax scale absorbs 1/S_C

    KHC = ((HIDDEN // N_CORES) if collective else HIDDEN) // 128

    def din(name, shape, dt=BF):
        return nc.dram_tensor(name, shape, dt, kind="ExternalInput").ap()

    d_hT = din("hT", [128, KHC, BSZ])
    d_wqaT = din("wqaT", [128, KHC, Q_LORA])
    d_wqbT = din("wqbT", [128, KQ, HP * Q_D], WQB_DT)
    d_wnope = din("wnope", [128, HP, KV_LORA])
    d_wvT = din("wvT", [128, ND, HP * V_D])
    d_woT = din("woT", [128, HP, HIDDEN], WO_DT)
    d_c = din("ckv", [BSZ, 128, NK1, KV_LORA], C_DT)
    d_cT = din("ckvT", [BSZ, 128, ND, KV_LEN], CT_DT)
    d_kpe = din("kpeT", [2, 128, KV_LEN])
    d_bmask = din("bmask", [BSZ, 128], F32)
    d_out = nc.dram_tensor("out_partial", [BSZ, HIDDEN], F32, kind="ExternalOutput").ap()

    with ExitStack() as ctx:
        tc = ctx.enter_context(tile.TileContext(nc))
        p1 = ctx.enter_context(tc.tile_pool(name="p1", bufs=1))        # consts+small
        pwqa = ctx.enter_context(tc.tile_pool(name="pwqa", bufs=2))    # 2x3K
        pw = ctx.enter_context(tc.tile_pool(name="pw", bufs=PW_BUFS))  # 4x6K
        pwo = ctx.enter_context(tc.tile_pool(name="pwo", bufs=PWO_BUFS))  # 4x5K
        pc = ctx.enter_context(tc.tile_pool(name="pc", bufs=PC_BUFS))  # 4x8K fp8
        pcT = ctx.enter_context(tc.tile_pool(name="pcT", bufs=PCT_BUFS))  # 4x8K fp8
        pkpe = ctx.enter_context(tc.tile_pool(name="pkpe", bufs=1))    # 8K
        pbig = ctx.enter_context(tc.tile_pool(name="pbig", bufs=1))    # attn 4K
        pm2 = ctx.enter_context(tc.tile_pool(name="pm2", bufs=2))      # ~12K
        pout = ctx.enter_context(tc.tile_pool(name="pout", bufs=1))    # 20K
        pdram = ctx.enter_context(tc.tile_pool(name="pdram", bufs=1, space="DRAM"))
        acc = ctx.enter_context(tc.tile_pool(name="acc", bufs=ACC_BUFS, space="PSUM"))
        tp = ctx.enter_context(tc.tile_pool(name="tp", bufs=TP_BUFS, space="PSUM"))

        def ps_acc(name="ps"):
            return acc.tile([128, 512], F32, tag="ps", name=name)

        def ps_tp4(dt=F32):
            return tp.tile([128, 512], dt, tag="tp4", name="tp4")

        if STAGE == 'dma':
            # DMA-floor probe: same bytes, consolidated into few big DMAs.
            junk = p1.tile([128, 1], F32, tag="junk", name="junk")
            zout = p1.tile([4, HIDDEN // 2], F32, tag="zout", name="zout")
            nc.vector.memset(zout, 0.0)

            def sink(t2d):
                p = t2d.shape[0]
                nc.vector.reduce_max(out=junk[:p, :], in_=t2d[:, :4],
                                     axis=mybir.AxisListType.X)

            for _rep in range(reps):
                hT_sb = p1.tile([128, KHC, BSZ], BF, tag="hT", name="hT")
                nc.sync.dma_start(out=hT_sb, in_=d_hT)
                sink(hT_sb[:, 0, :])
                wqa_sb = pw.tile([128, KHC, Q_LORA], BF, tag="wqa", name="wqa",
                                 bufs=1)
                nc.sync.dma_start(out=wqa_sb, in_=d_wqaT)
                sink(wqa_sb[:, 0, :])
                kpe_all = pkpe.tile([128, 2, KV_LEN], BF, tag="kpe", name="kpe")
                nc.sync.dma_start(out=kpe_all, in_=d_kpe.rearrange("g p k -> p g k"))
                sink(kpe_all[:, 0, :])
                cc_in = pdram.tile([4, Q_LORA], F32, tag="cc_in", name="cc_in")
                cc_out = pdram.tile([4, Q_LORA], F32, tag="cc_out", name="cc_out")
                qa_part = pm2.tile([4, Q_LORA], F32, tag="qa_part", name="qa_part",
                                   bufs=1)
                nc.vector.memset(qa_part, 0.0)
                nc.sync.dma_start(out=cc_in, in_=qa_part)
                nc.gpsimd.collective_compute(
                    "AllReduce", OP.add,
                    replica_groups=[list(range(N_CORES))],
                    ins=[cc_in[:, :]], outs=[cc_out[:, :]],
                )
                qa_full = pm2.tile([4, Q_LORA], F32, tag="qa_full", name="qa_full",
                                   bufs=1)
                nc.sync.dma_start(out=qa_full, in_=cc_out)
                sink(qa_full[:4, :])
                for b in range(BSZ):
                    c_sb = pc.tile([128, NK1, KV_LORA], C_DT, tag="c32", name="c32",
                                   bufs=1)
                    nc.sync.dma_start(out=c_sb, in_=d_c[b])
                    sink(c_sb[:, 0, :])
                    cT_sb = pcT.tile([128, ND, KV_LEN], CT_DT, tag="cT", name="cT",
                                     bufs=1)
                    nc.sync.dma_start(out=cT_sb, in_=d_cT[b])
                    sink(cT_sb[:, 0, :])
                for kg in range(2):
                    wqb_sb = pw.tile([128, KQ // 2, HP * Q_D], WQB_DT, tag="w",
                                     name="w", bufs=1)
                    nc.sync.dma_start(out=wqb_sb,
                                      in_=d_wqbT[:, kg * (KQ // 2):(kg + 1) * (KQ // 2), :])
                    sink(wqb_sb[:, 0, :])
                wn_sb = p1.tile([128, HP, KV_LORA], BF, tag="wn", name="wn")
                nc.sync.dma_start(out=wn_sb, in_=d_wnope)
                sink(wn_sb[:, 0, :])
                wv_sb = p1.tile([128, ND, HP * V_D], BF, tag="wv", name="wv")
                nc.sync.dma_start(out=wv_sb, in_=d_wvT)
                sink(wv_sb[:, 0, :])
                for og in range(4):
                    wt = pwo.tile([128, 4, HIDDEN], WO_DT, tag="wo", name="wo",
                                  bufs=2)
                    nc.sync.dma_start(out=wt, in_=d_woT[:, og * 4:(og + 1) * 4, :])
                    sink(wt[:, 0, :])
                for hh in range(2):
                    nc.sync.dma_start(
                        out=d_out[:, hh * (HIDDEN // 2):(hh + 1) * (HIDDEN // 2)],
                        in_=zout)
            return nc

        for _rep in range(reps):
            # ---- constants ----
            ident = p1.tile([128, 128], F32, tag="ident", name="ident")
            make_identity(nc, ident)
            identR = p1.tile([128, 128], BF, tag="identR", name="identR")
            nc.vector.tensor_copy(out=identR, in_=ident)
            s2 = S_WQB * S_WQB if FP8_WQB else 1.0
            eps_sb = p1.tile([4, 1], F32, tag="eps", name="eps")
            nc.vector.memset(eps_sb, EPS * s2)
            bmask_sb = p1.tile([BSZ, 128], F32, tag="bmask", name="bmask")
            nc.sync.dma_start(out=bmask_sb, in_=d_bmask)
            hT_sb = p1.tile([128, KHC, BSZ], BF, tag="hT", name="hT")
            nc.sync.dma_start(out=hT_sb, in_=d_hT)

            # ---- q_a = hidden @ w_q_a.T -> [4, 1536], col-tiled 3 wide ----
            qa_ps = ps_acc("qa_ps")
            for k in range(KHC):
                wt = pwqa.tile([128, Q_LORA], BF, tag="wqa", name="wqa")
                nc.sync.dma_start(out=wt, in_=d_wqaT[:, k, :])
                for n in range(3):
                    nc.tensor.matmul(
                        qa_ps[32 * n:32 * n + 4, :], hT_sb[:, k, :],
                        wt[:, n * 512:(n + 1) * 512],
                        start=(k == 0), stop=(k == KHC - 1),
                        tile_position=(0, 32 * n), skip_group_check=True,
                    )

            # ---- wqb chunks 0..PW_BUFS-1 prefetch (rest issued later) ----
            wqb_sbs = {}

            def load_wqb(k):
                wqb_sbs[k] = pw.tile([128, HP * Q_D], WQB_DT, tag="w", name="w")
                nc.sync.dma_start(out=wqb_sbs[k], in_=d_wqbT[:, k, :])

            for k in range(PW_BUFS):
                load_wqb(k)

            # ---- q_a AllReduce across cores ----
            if collective:
                qa_part = pm2.tile([4, Q_LORA], F32, tag="qa_part", name="qa_part",
                                   bufs=1)
                for n in range(3):
                    nc.scalar.copy(out=qa_part[:, n * 512:(n + 1) * 512],
                                   in_=qa_ps[32 * n:32 * n + 4, :])
                cc_in = pdram.tile([4, Q_LORA], F32, tag="cc_in", name="cc_in")
                cc_out = pdram.tile([4, Q_LORA], F32, tag="cc_out", name="cc_out")
                nc.sync.dma_start(out=cc_in, in_=qa_part)
                groups = ([[i] for i in range(N_CORES)] if CC_SINGLETON
                          else [list(range(N_CORES))])
                nc.gpsimd.collective_compute(
                    "AllReduce", OP.add,
                    replica_groups=groups,
                    ins=[cc_in[:, :]], outs=[cc_out[:, :]],
                )

            # ---- kv-side loads (ordered by consumption time) ----
            cT_sbs, c_sbs = {}, {}
            for b in range(BSZ):
                cT_sbs[b] = pcT.tile([128, ND, KV_LEN], CT_DT, tag="cT", name="cT")
                nc.sync.dma_start(out=cT_sbs[b], in_=d_cT[b])
            for b in range(BSZ):
                c_sbs[b] = pc.tile([128, NK1, KV_LORA], C_DT, tag="c32", name="c32")
                nc.sync.dma_start(out=c_sbs[b], in_=d_c[b])
            wn_sb = p1.tile([128, HP, KV_LORA], BF, tag="wn", name="wn")
            nc.sync.dma_start(out=wn_sb, in_=d_wnope)

            for k in range(PW_BUFS, KQ):
                load_wqb(k)
            wv_sb = p1.tile([128, ND, HP * V_D], BF, tag="wv", name="wv")
            nc.sync.dma_start(out=wv_sb, in_=d_wvT)
            kpe_all = pkpe.tile([128, 2, KV_LEN], BF, tag="kpe", name="kpe")
            nc.sync.dma_start(out=kpe_all, in_=d_kpe.rearrange("g p k -> p g k"))

            # qa_full read goes via the SWDGE (gpsimd) DMA ring so its wait
            # on the collective cannot head-of-line-block the SP DMA FIFO
            # that streams the weights
            qa_full = pm2.tile([4, Q_LORA], F32, tag="qa_full", name="qa_full",
                               bufs=1)
            if collective:
                nc.gpsimd.dma_start(out=qa_full, in_=cc_out)
            else:
                for n in range(3):
                    nc.scalar.copy(out=qa_full[:, n * 512:(n + 1) * 512],
                                   in_=qa_ps[32 * n:32 * n + 4, :])

            wo_sbs = {}
            for cc in range(HP):
                wo_sbs[cc] = pwo.tile([128, HIDDEN], WO_DT, tag="wo", name="wo")
                nc.sync.dma_start(out=wo_sbs[cc], in_=d_woT[:, cc, :])

            # ---- rstd = 1/rms(q_a) -> per-partition softmax scale ----
            sqs = [p1.tile([4, 1], F32, tag=f"sqs{n}", name=f"sqs{n}") for n in range(3)]
            sq = pm2.tile([4, 512], F32, tag="sq", name="sq", bufs=1)
            for n in range(3):
                nc.scalar.activation(out=sq, in_=qa_full[:, n * 512:(n + 1) * 512],
                                     func=ACTF.Square, accum_out=sqs[n])
            ssum = p1.tile([4, 1], F32, tag="ssum", name="ssum")
            nc.vector.tensor_tensor(out=ssum, in0=sqs[0], in1=sqs[1], op=OP.add)
            nc.vector.tensor_tensor(out=ssum, in0=sqs[2], in1=ssum, op=OP.add)
            rstd = p1.tile([4, 1], F32, tag="rstd", name="rstd")
            nc.scalar.activation(out=rstd, in_=ssum, func=ACTF.Sqrt, bias=eps_sb,
                                 scale=s2 / Q_LORA)
            nc.vector.reciprocal(out=rstd, in_=rstd)
            # broadcast to psum-partition layout: scale_ap[32b+j] = rstd[b]*SC_EFF
            rb_ps = tp.tile([128, 512], F32, tag="tp4", name="rb_ps")
            nc.tensor.matmul(rb_ps[:, :1], bmask_sb, rstd, start=True, stop=True)
            scale_ap = p1.tile([128, 1], F32, tag="scale_ap", name="scale_ap")
            nc.scalar.mul(out=scale_ap, in_=rb_ps[:, :1], mul=SC_EFF)

            # ---- transpose raw q_a -> qaT, interleaved with q_b matmuls ----
            # bank A: nope chunks 0-3 (heads 4j..4j+3); bank B: pe chunks 0-1
            qaT = p1.tile([128, KQ, 4], BF, tag="qaT", name="qaT")
            qbA = ps_acc("qbA")
            qbB = ps_acc("qbB")
            for kb in range(KQ // 4):
                pt = ps_tp4()
                for j in range(4):
                    k = kb * 4 + j
                    nc.tensor.transpose(pt[:, j * 128:j * 128 + 4],
                                        qa_full[:, k * 128:(k + 1) * 128],
                                        ident[:4, :4])
                nc.scalar.copy(out=qaT[:, kb * 4:(kb + 1) * 4, :],
                               in_=pt.rearrange("p (j x) -> p j x", x=128)[:, :, :4])
                for j in range(4):
                    k = kb * 4 + j
                    wt = wqb_sbs[k]
                    for n in range(4):
                        nc.tensor.matmul(
                            qbA[32 * n:32 * n + 4, :], qaT[:, k, :],
                            wt[:, n * 512:(n + 1) * 512],
                            start=(k == 0), stop=(k == KQ - 1),
                            tile_position=(0, 32 * n), skip_group_check=True,
                        )
                    for g in range(2):
                        nc.tensor.matmul(
                            qbB[32 * g:32 * g + 4, :], qaT[:, k, :],
                            wt[:, 2048 + g * 512:2048 + (g + 1) * 512],
                            start=(k == 0), stop=(k == KQ - 1),
                            tile_position=(0, 32 * g), skip_group_check=True,
                        )
            qsbA = p1.tile([128, 512], BF, tag="qsbA", name="qsbA")
            qsbB = p1.tile([128, 512], BF, tag="qsbB", name="qsbB")
            for n in range(4):
                nc.scalar.copy(out=qsbA[32 * n:32 * n + 4, :],
                               in_=qbA[32 * n:32 * n + 4, :])
            for g in range(2):
                nc.scalar.copy(out=qsbB[32 * g:32 * g + 4, :],
                               in_=qbB[32 * g:32 * g + 4, :])

            # ---- per-head transposes: q_nope -> qnT [128, 16h, 4b] ----
            qnT = p1.tile([128, HP, 4], BF, tag="qnT", name="qnT")
            ptn = ps_tp4(BF)
            for j in range(4):
                for i in range(4):
                    h = 4 * j + i
                    nc.tensor.transpose(ptn[:, h * 4:h * 4 + 4],
                                        qsbA[32 * j:32 * j + 4, i * 128:(i + 1) * 128],
                                        identR[32 * j:32 * j + 4, 32 * j:32 * j + 4],
                                        tile_position=(32 * j, 0))
            nc.vector.tensor_copy(out=qnT,
                                  in_=ptn.rearrange("p (h x) -> p h x", x=4)[:, :HP, :])

            # ---- q_pe -> qpe64 [64, 128] cols 32b+h (zero-padded) ----
            qpe64 = p1.tile([64, 128], BF, tag="qpe64", name="qpe64")
            nc.vector.memset(qpe64, 0.0)
            ptp = ps_tp4(BF)
            for g in range(2):
                for u in range(8):
                    h = 8 * g + u
                    nc.tensor.transpose(ptp[:64, h * 4:h * 4 + 4],
                                        qsbB[32 * g:32 * g + 4, u * 64:(u + 1) * 64],
                                        identR[32 * g:32 * g + 4, 32 * g:32 * g + 4],
                                        tile_position=(32 * g, 0))
            nc.vector.tensor_copy(
                out=qpe64.rearrange("p (b s) -> p s b", s=32)[:, :HP, :],
                in_=ptp[:64].rearrange("p (h b) -> p h b", b=4)[:, :HP, :])
            qpeT4 = p1.tile([128, 128], BF, tag="qpeT4", name="qpeT4")
            for bb in range(2):
                nc.sync.dma_start(out=qpeT4[bb * 64:(bb + 1) * 64], in_=qpe64)

            # ---- packed scores, pipelined with q_eff production ----
            # psum rows 32b + h, col-tiled over batches; b innermost so
            # adjacent matmuls hit different col-groups and overlap in the
            # array (pc-monotone starts would otherwise serialize chains).
            # pe-part first (needs only qpeT4); then each dd round produces
            # qeT[:, dd] = W_nope[h].T @ q_nope[h] and immediately streams
            # it against all four batches' cT.
            qeT = p1.tile([128, ND, 128], BF, tag="qeT", name="qeT")
            nc.vector.memset(qeT, 0.0)
            qeT_v = qeT.rearrange("p d (b s) -> p d s b", s=32)
            s_ps = [ps_acc(f"s{n}") for n in range(NK5)]
            for n in range(NK5):
                for b in range(BSZ):
                    hb = 64 * (b % 2)
                    nc.tensor.matmul(
                        s_ps[n][32 * b:32 * b + 32, :],
                        qpeT4[hb:hb + 64, 32 * b:32 * b + 32],
                        kpe_all[hb:hb + 64, b // 2, n * 512:(n + 1) * 512],
                        start=True, stop=False,
                        tile_position=(hb, 32 * b), skip_group_check=True,
                    )
            for dd in range(ND):
                qe_ps = ps_acc("qe_ps")
                for h in range(HP):
                    nc.tensor.matmul(qe_ps[:, h * 4:(h + 1) * 4],
                                     wn_sb[:, h, dd * 128:(dd + 1) * 128],
                                     qnT[:, h, :], start=True, stop=True,
                                     skip_group_check=True)
                nc.vector.tensor_copy(
                    out=qeT_v[:, dd, :HP, :],
                    in_=qe_ps[:, :64].rearrange("p (h b) -> p h b", b=4))
                for n in range(NK5):
                    for b in range(BSZ):
                        nc.tensor.matmul(
                            s_ps[n][32 * b:32 * b + 32, :],
                            qeT[:, dd, 32 * b:32 * b + 32],
                            cT_sbs[b][:, dd, n * 512:(n + 1) * 512],
                            start=False, stop=(dd == ND - 1),
                            tile_position=(0, 32 * b), skip_group_check=True,
                        )

            # ---- packed softmax (rmsnorm folded in via scale_ap) ----
            mxs = p1.tile([128, NK5], F32, tag="mxs", name="mxs")
            for n in range(NK5):
                nc.vector.reduce_max(out=mxs[:, n:n + 1], in_=s_ps[n], axis=AX.X)
            nmx = p1.tile([128, 1], F32, tag="nmx", name="nmx")
            nc.vector.reduce_max(out=nmx, in_=mxs, axis=AX.X, negate=True)
            nc.vector.tensor_tensor(out=nmx, in0=nmx, in1=scale_ap, op=OP.mult)
            attn = pbig.tile([128, KV_LEN], BF, tag="attn", name="attn")
            esums = p1.tile([128, NK5], F32, tag="esums", name="esums")
            for n in range(NK5):
                nc.scalar.activation(out=attn[:, n * 512:(n + 1) * 512],
                                     in_=s_ps[n], func=ACTF.Exp, bias=nmx,
                                     scale=scale_ap, accum_out=esums[:, n:n + 1])
            esum = p1.tile([128, 1], F32, tag="esum", name="esum")
            nc.vector.reduce_sum(out=esum, in_=esums, axis=AX.X)
            rsum = p1.tile([128, 1], F32, tag="rsum", name="rsum")
            nc.vector.reciprocal(out=rsum, in_=esum)
            if FP8_C:
                nc.vector.tensor_scalar_mul(out=rsum, in0=rsum, scalar1=1.0 / S_C)

            # ---- attnT transposes pipelined with o_c = attn @ c ----
            # col-tiled over batches; psum rows 32b+h
            attnT = pm2.tile([128, NK1, 128], BF, tag="attnT", name="attnT")
            oc_ps = ps_acc("oc_ps")
            for ob in range(NK1 // 4):
                pt = ps_tp4(BF)
                for j in range(4):
                    o = ob * 4 + j
                    nc.tensor.transpose(pt[:, j * 128:(j + 1) * 128],
                                        attn[:, o * 128:(o + 1) * 128],
                                        identR)
                nc.vector.tensor_copy(
                    out=attnT[:, ob * 4:(ob + 1) * 4, :],
                    in_=pt.rearrange("p (j x) -> p j x", x=128))
                for j in range(4):
                    o = ob * 4 + j
                    for b in range(BSZ):
                        nc.tensor.matmul(
                            oc_ps[32 * b:32 * b + 32, :],
                            attnT[:, o, 32 * b:32 * b + 32],
                            c_sbs[b][:, o, :],
                            start=(o == 0), stop=(o == NK1 - 1),
                            tile_position=(0, 32 * b), skip_group_check=True,
                        )
            oc_sb = p1.tile([128, KV_LORA], BF, tag="oc_sb", name="oc_sb")
            nc.vector.tensor_scalar_mul(out=oc_sb, in0=oc_ps, scalar1=rsum)

            # ---- ocT [512d, 128(32b+h)] via 4 transposes ----
            ocT = p1.tile([128, ND, 128], BF, tag="ocT", name="ocT")
            pt = ps_tp4(BF)
            for dd in range(ND):
                nc.tensor.transpose(pt[:, dd * 128:(dd + 1) * 128],
                                    oc_sb[:, dd * 128:(dd + 1) * 128], identR)
            nc.vector.tensor_copy(out=ocT,
                                  in_=pt.rearrange("p (d x) -> p d x", x=128))

            # ---- out_v pipelined with o_proj (head h feeds o_proj cc=h) ----
            yT = p1.tile([128, HP * BSZ], BF, tag="yT", name="yT")
            ocT_v = ocT.rearrange("p d (b s) -> p d s b", s=32)
            y_ps = ps_acc("y_ps")
            o_ps = [ps_acc(f"o{g}") for g in range(3)]
            for h in range(HP):
                for dd in range(ND):
                    nc.tensor.matmul(
                        y_ps[:, h * 4:(h + 1) * 4],
                        wv_sb[:, dd, h * V_D:(h + 1) * V_D],
                        ocT_v[:, dd, h, :],
                        start=(dd == 0), stop=(dd == ND - 1),
                        skip_group_check=True,
                    )
                nc.vector.tensor_copy(out=yT[:, h * 4:(h + 1) * 4],
                                      in_=y_ps[:, h * 4:(h + 1) * 4])
                wt = wo_sbs[h]
                for e in range(HIDDEN // 512):
                    g, j = divmod(e, 4)
                    nc.tensor.matmul(
                        o_ps[g][32 * j:32 * j + 4, :],
                        yT[:, h * BSZ:(h + 1) * BSZ],
                        wt[:, e * 512:(e + 1) * 512],
                        start=(h == 0), stop=(h == HP - 1),
                        tile_position=(0, 32 * j), skip_group_check=True,
                    )
            out_sb = pout.tile([4, HIDDEN], F32, tag="out_sb", name="out_sb")
            for e in range(HIDDEN // 512):
                g, j = divmod(e, 4)
                src = o_ps[g][32 * j:32 * j + 4, :]
                dst = out_sb[:, e * 512:(e + 1) * 512]
                if e % 2 == 0:
                    if FP8_WO:
                        nc.scalar.mul(out=dst, in_=src, mul=1.0 / S_WO)
                    else:
                        nc.scalar.copy(out=dst, in_=src)
                else:
                    if FP8_WO:
                        nc.vector.tensor_scalar_mul(out=dst, in0=src,
                                                    scalar1=1.0 / S_WO)
                    else:
                        nc.vector.tensor_copy(out=dst, in_=src)
            nc.sync.dma_start(out=d_out, in_=out_sb)

    return nc


def _build(reps=1):
    key = ("nc", reps, COLLECTIVE, STAGE, FP8_CT, FP8_WO, FP8_C, FP8_WQB)
    if key not in _BUILD_CACHE:
        from concourse import bacc
        nc = bacc.Bacc("TRN2", target_bir_lowering=False, debug=False,
                       num_devices=N_CORES)
        _emit_kernel(nc, reps=reps, collective=COLLECTIVE)
        nc.compile()
        _BUILD_CACHE[key] = nc
    return _BUILD_CACHE[key]


# --------------------------------------------------------------------------
# entry point
# --------------------------------------------------------------------------

def _run(inputs, **kw):
    from concourse.bass_utils import run_bass_kernel_spmd

    nc = _build()
    shared = _prep_shared(inputs)
    in_maps = [_prep_core(inputs, shared, c) for c in range(N_CORES)]
    res = run_bass_kernel_spmd(nc, in_maps, core_ids=list(range(N_CORES)), **kw)
    out = np.zeros((BSZ, HIDDEN), np.float32)
    for r in res.results:
        out += r["out_partial"]
    return out.reshape(BSZ, 1, HIDDEN), res


def kernel(**inputs):
    out, _ = _run(inputs)
    return out


def run_with_trace(inputs, **kw):
    """test-harness helper: returns (output, BassKernelResults)."""
    return _run(inputs, **kw)


# revision 55
# speedup vs baseline: 1.1664x; 1.1664x over previous
"""DeepseekV2 MLA decode attention (bsz=4, q_len=1, kv_len=2048) on 8 TRN2 cores.

Sharding: tensor-parallel over the 128 heads (16 heads/core).
  - w_q_b / w_kv_b output heads and w_o input heads are sharded.
  - w_q_a is column-sharded (hidden dim) with an on-device AllReduce of the
    tiny [4, 1536] q_a partial (COLLECTIVE=True); KV caches are replicated.
  - Each core computes a partial o_proj output [4, 5120]; the host sums the
    8 partials (the all-reduce of column-parallel o_proj).

Math restructuring (exact up to fp rounding):
  - "Absorbed" MLA: q_eff = W_kv_nope[h].T @ q_nope, scores_nope = q_eff . c
    and o_c = attn @ c, out_v = W_v[h] @ o_c.
  - RoPE folded into host prep (k_pe rotated per-position on host; the
    q-side last-position rotation is folded into w_q_b's pe rows).
  - rmsnorm folded into softmax: scores are computed from the RAW q_a
    (rmsnorm is a per-(batch) positive scale on q, linear through q_b /
    q_eff / scores), and 1/rms enters as a per-partition scale AP on the
    softmax exp. This removes the qan materialization entirely.
  - fp8 (e3m4) streaming for ckv/ckvT/w_o halves their HBM bytes; descales
    are exact powers of two folded into existing constants.
  - Batch-packed attention: partitions 32*b + h hold (batch b, head h) for
    scores / softmax / attn / o_c, so the four batches' small-M matmuls run
    col-tiled (tile_position) in the four 32-col groups of the PE array
    concurrently, and softmax ops process all batches in one [128, 512] op.
  - All DMAs are issued in consumption order up front; the fabric is
    byte-bound (~430 GB/s measured), so the kernel is designed to keep the
    single FIFO DMA stream dense and hide compute under it, with w_o
    (largest, needed last) streamed at the tail.
"""

import numpy as np
from contextlib import ExitStack

HIDDEN = 5120
NUM_HEADS = 128
Q_LORA = 1536
ROPE_D = 64
KV_LORA = 512
V_D = 128
NOPE_D = 128
Q_D = 192
THETA = 10000.0
EPS = 1e-6

N_CORES = 8
HP = NUM_HEADS // N_CORES  # 16 heads per core
BSZ = 4
KV_LEN = 2048

KQ = Q_LORA // 128   # 12
NK5 = KV_LEN // 512  # 4
NK1 = KV_LEN // 128  # 16
ND = KV_LORA // 128  # 4
SCALE = float(Q_D) ** -0.5

COLLECTIVE = True
CC_SINGLETON = False  # timing probe: per-core singleton AllReduce (WRONG output)
STAGE = 'all'  # 'all' | 'dma' (DMA-floor probe: loads only, no compute)

# fp8 (e3m4) streaming for the big HBM tensors. Descale factors are exact
# powers of two folded into existing constants (softmax scale, rmsnorm eps,
# psum-evacuation scales), so the math matches bf16 up to quantization noise.
FP8_CT = True    # ckvT (scores operand): -4.2 MB/core, ~+0.6e-2 rel err
FP8_WO = True    # w_o: -10.5 MB/core, ~+1.1e-2
FP8_C = True     # ckv (o_c operand): -4.2 MB/core, ~+1.0e-2
FP8_WQB = True   # w_q_b: -4.7 MB/core, ~+1.0e-2
S_C = 2.0        # ckv/ckvT/kpe pre-scale (max |ckv| ~5.2 -> 10.4 < 15.5)
S_WO = 128.0     # w_o pre-scale (sigma 0.02 -> 2.56)
S_WQB = 128.0    # w_q_b pre-scale

PW_BUFS = 8    # wqb stream pool (chunks 8-11 issued late in the FIFO)
PWO_BUFS = 7   # w_o stream pool (deeper = shorter DMA tail after o_proj start)
PC_BUFS = 4    # all four batches' c resident (packed o_c)
PCT_BUFS = 4   # all four batches' cT resident (packed scores)
ACC_BUFS = 6
TP_BUFS = 2

_BUILD_CACHE = {}


# --------------------------------------------------------------------------
# host-side prep
# --------------------------------------------------------------------------

def _bf16(x):
    import ml_dtypes
    return np.ascontiguousarray(np.asarray(x, np.float32).astype(ml_dtypes.bfloat16))


def _e3m4(x, scale):
    import ml_dtypes
    xs = np.clip(np.asarray(x, np.float32) * np.float32(scale), -15.5, 15.5)
    return np.ascontiguousarray(xs.astype(ml_dtypes.float8_e3m4))


def _rope_tables():
    exps = np.arange(0, ROPE_D, 2, dtype=np.float32) / np.float32(ROPE_D)
    inv_freq = (np.float32(1.0) / (np.float32(THETA) ** exps)).astype(np.float32)
    ang = np.arange(KV_LEN, dtype=np.float32)[:, None] * inv_freq[None, :]
    cos = np.cos(ang).astype(np.float32)  # [kv, 32]
    sin = np.sin(ang).astype(np.float32)
    return cos, sin


def _swiz(a, p=128):
    """[O*p, N] row-major -> [p, O*N] p-major (row o*p+q lands at [q, o*N:])."""
    o = a.shape[0] // p
    return np.ascontiguousarray(
        a.reshape(o, p, a.shape[1]).transpose(1, 0, 2).reshape(p, o * a.shape[1]))


def _prep_shared(inputs):
    """Host prep shared across cores (replicated tensors)."""
    hidden = np.asarray(inputs["hidden_states"], np.float32)
    ckv = np.asarray(inputs["compressed_kv_normed_cache"], np.float32)
    kpe = np.asarray(inputs["k_pe_cache"], np.float32)
    wqa = np.asarray(inputs["w_q_a"], np.float32)

    hT = np.ascontiguousarray(hidden[:, 0, :].T)           # [5120, 4]
    wqaT = np.ascontiguousarray(wqa.T)                     # [5120, 1536]; sliced per core

    cos, sin = _rope_tables()
    # rotate k_pe on host (per-position rope applied to the cache) and
    # de-interleave: group g={b01,b23}, partition (b%2)*64 + t*32 + f
    kr = kpe[:, :, 0::2]                                   # [4, kv, 32]
    ki = kpe[:, :, 1::2]
    rr = kr * cos[None] - ki * sin[None]
    ri = kr * sin[None] + ki * cos[None]
    k64 = np.concatenate([rr.transpose(0, 2, 1), ri.transpose(0, 2, 1)], axis=1)
    k64 = k64.reshape(2, 128, KV_LEN)
    # with fp8 ckvT, the score psum is S_C*score; scale the kpe operand to
    # match and fold the 1/S_C into the softmax scale constant
    kpeT = _bf16(k64 * S_C) if FP8_CT else _bf16(k64)      # [2, 128, kv]

    # ckv: per batch p-major [128, 16, 512] (kv row o*128+p -> [p, o, :])
    ckv_s = np.stack([_swiz(ckv[b]) for b in range(BSZ)]).reshape(BSZ, 128, NK1, KV_LORA)
    # ckvT: per batch [128 (d%128), 4 (d//128), 2048 kv] — scores operand
    ckvT_s = np.stack([_swiz(np.ascontiguousarray(ckv[b].T)) for b in range(BSZ)])
    ckvT_s = ckvT_s.reshape(BSZ, 128, ND, KV_LEN)

    # batch-broadcast mask: bmask[b, 32b:32b+32] = 1 (psum-partition layout)
    bmask = np.zeros((BSZ, 128), np.float32)
    for b in range(BSZ):
        bmask[b, 32 * b:32 * (b + 1)] = 1.0

    return dict(hT=hT, wqaT=wqaT,
                ckv=_e3m4(ckv_s, S_C) if FP8_C else _bf16(ckv_s),
                ckvT=_e3m4(ckvT_s, S_C) if FP8_CT else _bf16(ckvT_s),
                kpeT=kpeT, bmask=np.ascontiguousarray(bmask),
                cosL=cos[-1], sinL=sin[-1])


def _prep_core(inputs, shared, core):
    wqb = np.asarray(inputs["w_q_b"], np.float32).reshape(NUM_HEADS, Q_D, Q_LORA)
    wkv = np.asarray(inputs["w_kv_b"], np.float32).reshape(NUM_HEADS, NOPE_D + V_D, KV_LORA)
    wo = np.asarray(inputs["w_o"], np.float32)
    ln = np.asarray(inputs["w_q_a_ln"], np.float32)
    cosL, sinL = shared["cosL"], shared["sinL"]            # [32]

    h0 = core * HP
    # per-head q rows: nope, then rotated+de-interleaved pe rows
    # (the last-position rope of q_pe is linear in q -> fold into w_q_b)
    wqb_c = wqb[h0:h0 + HP]                                # [16, 192, 1536]
    wn_rows = wqb_c[:, :NOPE_D, :]                         # [16, 128, 1536]
    wre = wqb_c[:, NOPE_D + 0::2, :]                       # [16, 32, 1536]
    wim = wqb_c[:, NOPE_D + 1::2, :]
    rot_re = cosL[None, :, None] * wre - sinL[None, :, None] * wim
    rot_im = sinL[None, :, None] * wre + cosL[None, :, None] * wim
    # column order: all heads' nope rows first (4 chunks of 4 heads), then
    # all heads' pe rows (2 chunks of 8 heads) — aligns 512-col psum chunks
    # with whole heads for the col-tiled q_b + row-tiled transposes
    pe_rows = np.concatenate([rot_re, rot_im], axis=1)     # [16, 64, 1536]
    wqb_r = np.concatenate([wn_rows.reshape(HP * NOPE_D, Q_LORA),
                            pe_rows.reshape(HP * 64, Q_LORA)], axis=0)
    wqb_r = wqb_r * ln[None, :]                            # [3072, 1536]
    wqbT = _swiz(np.ascontiguousarray(wqb_r.T))            # [128, 12*3072]

    wkv_c = wkv[h0:h0 + HP]                                # [16, 256, 512]
    wnope = _swiz(wkv_c[:, :NOPE_D, :].reshape(HP * NOPE_D, KV_LORA))  # [128,16*512]
    wv = wkv_c[:, NOPE_D:, :]                              # [16, 128, 512]
    wvT = _swiz(np.ascontiguousarray(
        wv.transpose(2, 0, 1).reshape(KV_LORA, HP * V_D)))  # [128, 4*2048]

    woT = np.ascontiguousarray(wo[:, h0 * V_D:(h0 + HP) * V_D].T)  # [2048, 5120]
    # p-major [128, 16, 5120]: partition p holds rows {o*128+p}, contiguous
    woT = np.ascontiguousarray(
        woT.reshape(HP, 128, HIDDEN).transpose(1, 0, 2))
    woT = _e3m4(woT, S_WO) if FP8_WO else _bf16(woT)

    m = dict(ckv=shared["ckv"], ckvT=shared["ckvT"], kpeT=shared["kpeT"],
             bmask=shared["bmask"])
    if COLLECTIVE:
        ksl = HIDDEN // N_CORES
        wqaT = shared["wqaT"][core * ksl:(core + 1) * ksl]
        hT = shared["hT"][core * ksl:(core + 1) * ksl]
    else:
        wqaT, hT = shared["wqaT"], shared["hT"]
    khc = wqaT.shape[0] // 128
    m["wqaT"] = _bf16(_swiz(wqaT)).reshape(128, khc, Q_LORA)
    m["hT"] = _bf16(_swiz(hT)).reshape(128, khc, BSZ)
    wqbT_m = (_e3m4(wqbT, S_WQB) if FP8_WQB else _bf16(wqbT))
    m.update(wqbT=wqbT_m.reshape(128, KQ, HP * Q_D),
             wnope=_bf16(wnope).reshape(128, HP, KV_LORA),
             wvT=_bf16(wvT).reshape(128, ND, HP * V_D),
             woT=woT)
    return m


# --------------------------------------------------------------------------
# device kernel
# --------------------------------------------------------------------------

def _emit_kernel(nc, reps=1, collective=COLLECTIVE):
    import concourse.tile as tile
    import concourse.mybir as mybir
    from concourse.masks import make_identity

    F32 = mybir.dt.float32
    BF = mybir.dt.bfloat16
    F8 = mybir.dt.float8e3
    AX = mybir.AxisListType
    OP = mybir.AluOpType
    ACTF = mybir.ActivationFunctionType

    CT_DT = F8 if FP8_CT else BF
    C_DT = F8 if FP8_C else BF
    WO_DT = F8 if FP8_WO else BF
    WQB_DT = F8 if FP8_WQB else BF
    SC_EFF = SCALE / S_C if FP8_CT else SCALE  # softmax scale absorbs 1/S_C

    KHC = ((HIDDEN // N_CORES) if collective else HIDDEN) // 128

    def din(name, shape, dt=BF):
        return nc.dram_tensor(name, shape, dt, kind="ExternalInput").ap()

    d_hT = din("hT", [128, KHC, BSZ])
    d_wqaT = din("wqaT", [128, KHC, Q_LORA])
    d_wqbT = din("wqbT", [128, KQ, HP * Q_D], WQB_DT)
    d_wnope = din("wnope", [128, HP, KV_LORA])
    d_wvT = din("wvT", [128, ND, HP * V_D])
    d_woT = din("woT", [128, HP, HIDDEN], WO_DT)
    d_c = din("ckv", [BSZ, 128, NK1, KV_LORA], C_DT)
    d_cT = din("ckvT", [BSZ, 128, ND, KV_LEN], CT_DT)
    d_kpe = din("kpeT", [2, 128, KV_LEN])
    d_bmask = din("bmask", [BSZ, 128], F32)
    d_out = nc.dram_tensor("out_partial", [BSZ, HIDDEN], F32, kind="ExternalOutput").ap()

    with ExitStack() as ctx:
        tc = ctx.enter_context(tile.TileContext(nc))
        p1 = ctx.enter_context(tc.tile_pool(name="p1", bufs=1))        # consts+small
        pwqa = ctx.enter_context(tc.tile_pool(name="pwqa", bufs=2))    # 2x3K
        pw = ctx.enter_context(tc.tile_pool(name="pw", bufs=PW_BUFS))  # 4x6K
        pwo = ctx.enter_context(tc.tile_pool(name="pwo", bufs=PWO_BUFS))  # 4x5K
        pc = ctx.enter_context(tc.tile_pool(name="pc", bufs=PC_BUFS))  # 4x8K fp8
        pcT = ctx.enter_context(tc.tile_pool(name="pcT", bufs=PCT_BUFS))  # 4x8K fp8
        pkpe = ctx.enter_context(tc.tile_pool(name="pkpe", bufs=1))    # 8K
        pbig = ctx.enter_context(tc.tile_pool(name="pbig", bufs=1))    # attn 4K
        pm2 = ctx.enter_context(tc.tile_pool(name="pm2", bufs=2))      # ~12K
        pout = ctx.enter_context(tc.tile_pool(name="pout", bufs=1))    # 20K
        pdram = ctx.enter_context(tc.tile_pool(name="pdram", bufs=1, space="DRAM"))
        acc = ctx.enter_context(tc.tile_pool(name="acc", bufs=ACC_BUFS, space="PSUM"))
        tp = ctx.enter_context(tc.tile_pool(name="tp", bufs=TP_BUFS, space="PSUM"))

        def ps_acc(name="ps"):
            return acc.tile([128, 512], F32, tag="ps", name=name)

        def ps_tp4(dt=F32):
            return tp.tile([128, 512], dt, tag="tp4", name="tp4")

        if STAGE == 'dma':
            # DMA-floor probe: same bytes, consolidated into few big DMAs.
            junk = p1.tile([128, 1], F32, tag="junk", name="junk")
            zout = p1.tile([4, HIDDEN // 2], F32, tag="zout", name="zout")
            nc.vector.memset(zout, 0.0)

            def sink(t2d):
                p = t2d.shape[0]
                nc.vector.reduce_max(out=junk[:p, :], in_=t2d[:, :4],
                                     axis=mybir.AxisListType.X)

            for _rep in range(reps):
                hT_sb = p1.tile([128, KHC, BSZ], BF, tag="hT", name="hT")
                nc.sync.dma_start(out=hT_sb, in_=d_hT)
                sink(hT_sb[:, 0, :])
                wqa_sb = pw.tile([128, KHC, Q_LORA], BF, tag="wqa", name="wqa",
                                 bufs=1)
                nc.sync.dma_start(out=wqa_sb, in_=d_wqaT)
                sink(wqa_sb[:, 0, :])
                kpe_all = pkpe.tile([128, 2, KV_LEN], BF, tag="kpe", name="kpe")
                nc.sync.dma_start(out=kpe_all, in_=d_kpe.rearrange("g p k -> p g k"))
                sink(kpe_all[:, 0, :])
                cc_in = pdram.tile([4, Q_LORA], F32, tag="cc_in", name="cc_in")
                cc_out = pdram.tile([4, Q_LORA], F32, tag="cc_out", name="cc_out")
                qa_part = pm2.tile([4, Q_LORA], F32, tag="qa_part", name="qa_part",
                                   bufs=1)
                nc.vector.memset(qa_part, 0.0)
                nc.sync.dma_start(out=cc_in, in_=qa_part)
                nc.gpsimd.collective_compute(
                    "AllReduce", OP.add,
                    replica_groups=[list(range(N_CORES))],
                    ins=[cc_in[:, :]], outs=[cc_out[:, :]],
                )
                qa_full = pm2.tile([4, Q_LORA], F32, tag="qa_full", name="qa_full",
                                   bufs=1)
                nc.sync.dma_start(out=qa_full, in_=cc_out)
                sink(qa_full[:4, :])
                for b in range(BSZ):
                    c_sb = pc.tile([128, NK1, KV_LORA], C_DT, tag="c32", name="c32",
                                   bufs=1)
                    nc.sync.dma_start(out=c_sb, in_=d_c[b])
                    sink(c_sb[:, 0, :])
                    cT_sb = pcT.tile([128, ND, KV_LEN], CT_DT, tag="cT", name="cT",
                                     bufs=1)
                    nc.sync.dma_start(out=cT_sb, in_=d_cT[b])
                    sink(cT_sb[:, 0, :])
                for kg in range(2):
                    wqb_sb = pw.tile([128, KQ // 2, HP * Q_D], WQB_DT, tag="w",
                                     name="w", bufs=1)
                    nc.sync.dma_start(out=wqb_sb,
                                      in_=d_wqbT[:, kg * (KQ // 2):(kg + 1) * (KQ // 2), :])
                    sink(wqb_sb[:, 0, :])
                wn_sb = p1.tile([128, HP, KV_LORA], BF, tag="wn", name="wn")
                nc.sync.dma_start(out=wn_sb, in_=d_wnope)
                sink(wn_sb[:, 0, :])
                wv_sb = p1.tile([128, ND, HP * V_D], BF, tag="wv", name="wv")
                nc.sync.dma_start(out=wv_sb, in_=d_wvT)
                sink(wv_sb[:, 0, :])
                for og in range(4):
                    wt = pwo.tile([128, 4, HIDDEN], WO_DT, tag="wo", name="wo",
                                  bufs=2)
                    nc.sync.dma_start(out=wt, in_=d_woT[:, og * 4:(og + 1) * 4, :])
                    sink(wt[:, 0, :])
                for hh in range(2):
                    nc.sync.dma_start(
                        out=d_out[:, hh * (HIDDEN // 2):(hh + 1) * (HIDDEN // 2)],
                        in_=zout)
            return nc

        for _rep in range(reps):
            # ---- constants ----
            ident = p1.tile([128, 128], F32, tag="ident", name="ident")
            make_identity(nc, ident)
            identR = p1.tile([128, 128], BF, tag="identR", name="identR")
            nc.vector.tensor_copy(out=identR, in_=ident)
            s2 = S_WQB * S_WQB if FP8_WQB else 1.0
            eps_sb = p1.tile([4, 1], F32, tag="eps", name="eps")
            nc.vector.memset(eps_sb, EPS * s2)
            bmask_sb = p1.tile([BSZ, 128], F32, tag="bmask", name="bmask")
            nc.sync.dma_start(out=bmask_sb, in_=d_bmask)
            hT_sb = p1.tile([128, KHC, BSZ], BF, tag="hT", name="hT")
            nc.sync.dma_start(out=hT_sb, in_=d_hT)

            # ---- q_a = hidden @ w_q_a.T -> [4, 1536], col-tiled 3 wide ----
            qa_ps = ps_acc("qa_ps")
            for k in range(KHC):
                wt = pwqa.tile([128, Q_LORA], BF, tag="wqa", name="wqa")
                nc.sync.dma_start(out=wt, in_=d_wqaT[:, k, :])
                for n in range(3):
                    nc.tensor.matmul(
                        qa_ps[32 * n:32 * n + 4, :], hT_sb[:, k, :],
                        wt[:, n * 512:(n + 1) * 512],
                        start=(k == 0), stop=(k == KHC - 1),
                        tile_position=(0, 32 * n), skip_group_check=True,
                    )

            # ---- wqb chunks 0..PW_BUFS-1 prefetch (rest issued later) ----
            wqb_sbs = {}

            def load_wqb(k):
                wqb_sbs[k] = pw.tile([128, HP * Q_D], WQB_DT, tag="w", name="w")
                nc.sync.dma_start(out=wqb_sbs[k], in_=d_wqbT[:, k, :])

            for k in range(PW_BUFS):
                load_wqb(k)

            # ---- q_a AllReduce across cores ----
            if collective:
                qa_part = pm2.tile([4, Q_LORA], F32, tag="qa_part", name="qa_part",
                                   bufs=1)
                for n in range(3):
                    nc.scalar.copy(out=qa_part[:, n * 512:(n + 1) * 512],
                                   in_=qa_ps[32 * n:32 * n + 4, :])
                cc_in = pdram.tile([4, Q_LORA], F32, tag="cc_in", name="cc_in")
                cc_out = pdram.tile([4, Q_LORA], F32, tag="cc_out", name="cc_out")
                nc.sync.dma_start(out=cc_in, in_=qa_part)
                groups = ([[i] for i in range(N_CORES)] if CC_SINGLETON
                          else [list(range(N_CORES))])
                nc.gpsimd.collective_compute(
                    "AllReduce", OP.add,
                    replica_groups=groups,
                    ins=[cc_in[:, :]], outs=[cc_out[:, :]],
                )

            # ---- kv-side loads (ordered by consumption time) ----
            cT_sbs, c_sbs = {}, {}
            for b in range(BSZ):
                cT_sbs[b] = pcT.tile([128, ND, KV_LEN], CT_DT, tag="cT", name="cT")
                nc.sync.dma_start(out=cT_sbs[b], in_=d_cT[b])
            for b in range(BSZ):
                c_sbs[b] = pc.tile([128, NK1, KV_LORA], C_DT, tag="c32", name="c32")
                nc.sync.dma_start(out=c_sbs[b], in_=d_c[b])
            wn_sb = p1.tile([128, HP, KV_LORA], BF, tag="wn", name="wn")
            nc.sync.dma_start(out=wn_sb, in_=d_wnope)

            wv_sb = p1.tile([128, ND, HP * V_D], BF, tag="wv", name="wv")
            nc.sync.dma_start(out=wv_sb, in_=d_wvT)
            kpe_all = pkpe.tile([128, 2, KV_LEN], BF, tag="kpe", name="kpe")
            nc.sync.dma_start(out=kpe_all, in_=d_kpe.rearrange("g p k -> p g k"))
            for k in range(PW_BUFS, KQ):
                load_wqb(k)

            # qa_full read goes via the SWDGE (gpsimd) DMA ring so its wait
            # on the collective cannot head-of-line-block the SP DMA FIFO
            # that streams the weights
            qa_full = pm2.tile([4, Q_LORA], F32, tag="qa_part", name="qa_full",
                               bufs=1)
            if collective:
                nc.sync.dma_start(out=qa_full, in_=cc_out)
            else:
                for n in range(3):
                    nc.scalar.copy(out=qa_full[:, n * 512:(n + 1) * 512],
                                   in_=qa_ps[32 * n:32 * n + 4, :])

            wo_sbs = {}
            for cc in range(HP):
                wo_sbs[cc] = pwo.tile([128, HIDDEN], WO_DT, tag="wo", name="wo")
                nc.sync.dma_start(out=wo_sbs[cc], in_=d_woT[:, cc, :])

            # ---- rstd = 1/rms(q_a) -> per-partition softmax scale ----
            sqs = [p1.tile([4, 1], F32, tag=f"sqs{n}", name=f"sqs{n}") for n in range(3)]
            sq = pm2.tile([4, 512], F32, tag="sq", name="sq", bufs=1)
            for n in range(3):
                nc.scalar.activation(out=sq, in_=qa_full[:, n * 512:(n + 1) * 512],
                                     func=ACTF.Square, accum_out=sqs[n])
            ssum = p1.tile([4, 1], F32, tag="ssum", name="ssum")
            nc.vector.tensor_tensor(out=ssum, in0=sqs[0], in1=sqs[1], op=OP.add)
            nc.vector.tensor_tensor(out=ssum, in0=sqs[2], in1=ssum, op=OP.add)
            rstd = p1.tile([4, 1], F32, tag="rstd", name="rstd")
            nc.scalar.activation(out=rstd, in_=ssum, func=ACTF.Sqrt, bias=eps_sb,
                                 scale=s2 / Q_LORA)
            nc.vector.reciprocal(out=rstd, in_=rstd)
            # broadcast to psum-partition layout: scale_ap[32b+j] = rstd[b]*SC_EFF
            rb_ps = tp.tile([128, 512], F32, tag="tp4", name="rb_ps")
            nc.tensor.matmul(rb_ps[:, :1], bmask_sb, rstd, start=True, stop=True)
            scale_ap = p1.tile([128, 1], F32, tag="scale_ap", name="scale_ap")
            nc.scalar.mul(out=scale_ap, in_=rb_ps[:, :1], mul=SC_EFF)

            # ---- transpose raw q_a -> qaT, interleaved with q_b matmuls ----
            # bank A: nope chunks 0-3 (heads 4j..4j+3); bank B: pe chunks 0-1
            qaT = p1.tile([128, KQ, 4], BF, tag="qaT", name="qaT")
            qbA = ps_acc("qbA")
            qbB = ps_acc("qbB")
            for kb in range(KQ // 4):
                pt = ps_tp4()
                for j in range(4):
                    k = kb * 4 + j
                    nc.tensor.transpose(pt[:, j * 128:j * 128 + 4],
                                        qa_full[:, k * 128:(k + 1) * 128],
                                        ident[:4, :4])
                nc.scalar.copy(out=qaT[:, kb * 4:(kb + 1) * 4, :],
                               in_=pt.rearrange("p (j x) -> p j x", x=128)[:, :, :4])
                for j in range(4):
                    k = kb * 4 + j
                    wt = wqb_sbs[k]
                    for n in range(4):
                        nc.tensor.matmul(
                            qbA[32 * n:32 * n + 4, :], qaT[:, k, :],
                            wt[:, n * 512:(n + 1) * 512],
                            start=(k == 0), stop=(k == KQ - 1),
                            tile_position=(0, 32 * n), skip_group_check=True,
                        )
                    for g in range(2):
                        nc.tensor.matmul(
                            qbB[32 * g:32 * g + 4, :], qaT[:, k, :],
                            wt[:, 2048 + g * 512:2048 + (g + 1) * 512],
                            start=(k == 0), stop=(k == KQ - 1),
                            tile_position=(0, 32 * g), skip_group_check=True,
                        )
            qsbA = p1.tile([128, 512], BF, tag="qsbA", name="qsbA")
            qsbB = p1.tile([128, 512], BF, tag="qsbB", name="qsbB")
            for n in range(4):
                nc.scalar.copy(out=qsbA[32 * n:32 * n + 4, :],
                               in_=qbA[32 * n:32 * n + 4, :])
            for g in range(2):
                nc.scalar.copy(out=qsbB[32 * g:32 * g + 4, :],
                               in_=qbB[32 * g:32 * g + 4, :])

            # ---- per-head transposes: q_nope -> qnT [128, 16h, 4b] ----
            qnT = p1.tile([128, HP, 4], BF, tag="qnT", name="qnT")
            ptn = ps_tp4(BF)
            for j in range(4):
                for i in range(4):
                    h = 4 * j + i
                    nc.tensor.transpose(ptn[:, h * 4:h * 4 + 4],
                                        qsbA[32 * j:32 * j + 4, i * 128:(i + 1) * 128],
                                        identR[32 * j:32 * j + 4, 32 * j:32 * j + 4],
                                        tile_position=(32 * j, 0))
            nc.vector.tensor_copy(out=qnT,
                                  in_=ptn.rearrange("p (h x) -> p h x", x=4)[:, :HP, :])

            # ---- q_pe -> qpe64 [64, 128] cols 32b+h (zero-padded) ----
            qpe64 = p1.tile([64, 128], BF, tag="qpe64", name="qpe64")
            nc.vector.memset(qpe64, 0.0)
            ptp = ps_tp4(BF)
            for g in range(2):
                for u in range(8):
                    h = 8 * g + u
                    nc.tensor.transpose(ptp[:64, h * 4:h * 4 + 4],
                                        qsbB[32 * g:32 * g + 4, u * 64:(u + 1) * 64],
                                        identR[32 * g:32 * g + 4, 32 * g:32 * g + 4],
                                        tile_position=(32 * g, 0))
            nc.vector.tensor_copy(
                out=qpe64.rearrange("p (b s) -> p s b", s=32)[:, :HP, :],
                in_=ptp[:64].rearrange("p (h b) -> p h b", b=4)[:, :HP, :])
            qpeT4 = p1.tile([128, 128], BF, tag="qpeT4", name="qpeT4")
            for bb in range(2):
                nc.sync.dma_start(out=qpeT4[bb * 64:(bb + 1) * 64], in_=qpe64)

            # ---- packed scores, pipelined with q_eff production ----
            # psum rows 32b + h, col-tiled over batches; b innermost so
            # adjacent matmuls hit different col-groups and overlap in the
            # array (pc-monotone starts would otherwise serialize chains).
            # pe-part first (needs only qpeT4); then each dd round produces
            # qeT[:, dd] = W_nope[h].T @ q_nope[h] and immediately streams
            # it against all four batches' cT.
            qeT = p1.tile([128, ND, 128], BF, tag="qeT", name="qeT")
            nc.vector.memset(qeT, 0.0)
            qeT_v = qeT.rearrange("p d (b s) -> p d s b", s=32)
            s_ps = [ps_acc(f"s{n}") for n in range(NK5)]
            for n in range(NK5):
                for b in range(BSZ):
                    hb = 64 * (b % 2)
                    nc.tensor.matmul(
                        s_ps[n][32 * b:32 * b + 32, :],
                        qpeT4[hb:hb + 64, 32 * b:32 * b + 32],
                        kpe_all[hb:hb + 64, b // 2, n * 512:(n + 1) * 512],
                        start=True, stop=False,
                        tile_position=(hb, 32 * b), skip_group_check=True,
                    )
            for dd in range(ND):
                qe_ps = ps_acc("qe_ps")
                for h in range(HP):
                    nc.tensor.matmul(qe_ps[:, h * 4:(h + 1) * 4],
                                     wn_sb[:, h, dd * 128:(dd + 1) * 128],
                                     qnT[:, h, :], start=True, stop=True,
                                     skip_group_check=True)
                nc.vector.tensor_copy(
                    out=qeT_v[:, dd, :HP, :],
                    in_=qe_ps[:, :64].rearrange("p (h b) -> p h b", b=4))
                for n in range(NK5):
                    for b in range(BSZ):
                        nc.tensor.matmul(
                            s_ps[n][32 * b:32 * b + 32, :],
                            qeT[:, dd, 32 * b:32 * b + 32],
                            cT_sbs[b][:, dd, n * 512:(n + 1) * 512],
                            start=False, stop=(dd == ND - 1),
                            tile_position=(0, 32 * b), skip_group_check=True,
                        )

            # ---- packed softmax (rmsnorm folded in via scale_ap) ----
            mxs = p1.tile([128, NK5], F32, tag="mxs", name="mxs")
            for n in range(NK5):
                nc.vector.reduce_max(out=mxs[:, n:n + 1], in_=s_ps[n], axis=AX.X)
            nmx = p1.tile([128, 1], F32, tag="nmx", name="nmx")
            nc.vector.reduce_max(out=nmx, in_=mxs, axis=AX.X, negate=True)
            nc.vector.tensor_tensor(out=nmx, in0=nmx, in1=scale_ap, op=OP.mult)
            attn = pbig.tile([128, KV_LEN], BF, tag="attn", name="attn")
            esums = p1.tile([128, NK5], F32, tag="esums", name="esums")
            for n in range(NK5):
                nc.scalar.activation(out=attn[:, n * 512:(n + 1) * 512],
                                     in_=s_ps[n], func=ACTF.Exp, bias=nmx,
                                     scale=scale_ap, accum_out=esums[:, n:n + 1])
            esum = p1.tile([128, 1], F32, tag="esum", name="esum")
            nc.vector.reduce_sum(out=esum, in_=esums, axis=AX.X)
            rsum = p1.tile([128, 1], F32, tag="rsum", name="rsum")
            nc.vector.reciprocal(out=rsum, in_=esum)
            if FP8_C:
                nc.vector.tensor_scalar_mul(out=rsum, in0=rsum, scalar1=1.0 / S_C)

            # ---- attnT transposes pipelined with o_c = attn @ c ----
            # col-tiled over batches; psum rows 32b+h
            attnT = pm2.tile([128, NK1, 128], BF, tag="attnT", name="attnT",
                             bufs=1)
            oc_ps = ps_acc("oc_ps")
            for ob in range(NK1 // 4):
                pt = ps_tp4(BF)
                for j in range(4):
                    o = ob * 4 + j
                    nc.tensor.transpose(pt[:, j * 128:(j + 1) * 128],
                                        attn[:, o * 128:(o + 1) * 128],
                                        identR)
                nc.vector.tensor_copy(
                    out=attnT[:, ob * 4:(ob + 1) * 4, :],
                    in_=pt.rearrange("p (j x) -> p j x", x=128))
                for j in range(4):
                    o = ob * 4 + j
                    for b in range(BSZ):
                        nc.tensor.matmul(
                            oc_ps[32 * b:32 * b + 32, :],
                            attnT[:, o, 32 * b:32 * b + 32],
                            c_sbs[b][:, o, :],
                            start=(o == 0), stop=(o == NK1 - 1),
                            tile_position=(0, 32 * b), skip_group_check=True,
                        )
            oc_sb = p1.tile([128, KV_LORA], BF, tag="oc_sb", name="oc_sb")
            nc.vector.tensor_scalar_mul(out=oc_sb, in0=oc_ps, scalar1=rsum)

            # ---- ocT [512d, 128(32b+h)] via 4 transposes ----
            ocT = p1.tile([128, ND, 128], BF, tag="ocT", name="ocT")
            pt = ps_tp4(BF)
            for dd in range(ND):
                nc.tensor.transpose(pt[:, dd * 128:(dd + 1) * 128],
                                    oc_sb[:, dd * 128:(dd + 1) * 128], identR)
            nc.vector.tensor_copy(out=ocT,
                                  in_=pt.rearrange("p (d x) -> p d x", x=128))

            # ---- out_v pipelined with o_proj (head h feeds o_proj cc=h) ----
            yT = p1.tile([128, HP * BSZ], BF, tag="yT", name="yT")
            ocT_v = ocT.rearrange("p d (b s) -> p d s b", s=32)
            y_ps = ps_acc("y_ps")
            o_ps = [ps_acc(f"o{g}") for g in range(3)]
            for h in range(HP):
                for dd in range(ND):
                    nc.tensor.matmul(
                        y_ps[:, h * 4:(h + 1) * 4],
                        wv_sb[:, dd, h * V_D:(h + 1) * V_D],
                        ocT_v[:, dd, h, :],
                        start=(dd == 0), stop=(dd == ND - 1),
                        skip_group_check=True,
                    )
                nc.vector.tensor_copy(out=yT[:, h * 4:(h + 1) * 4],
                                      in_=y_ps[:, h * 4:(h + 1) * 4])
                wt = wo_sbs[h]
                for e in range(HIDDEN // 512):
                    g, j = divmod(e, 4)
                    nc.tensor.matmul(
                        o_ps[g][32 * j:32 * j + 4, :],
                        yT[:, h * BSZ:(h + 1) * BSZ],
                        wt[:, e * 512:(e + 1) * 512],
                        start=(h == 0), stop=(h == HP - 1),
                        tile_position=(0, 32 * j), skip_group_check=True,
                    )
            for hh in range(2):
                out_sb = pout.tile([4, HIDDEN // 2], F32, tag="out_sb",
                                   name="out_sb")
                for ee in range(HIDDEN // 1024):
                    e = hh * (HIDDEN // 1024) + ee
                    g, j = divmod(e, 4)
                    src = o_ps[g][32 * j:32 * j + 4, :]
                    dst = out_sb[:, ee * 512:(ee + 1) * 512]
                    if ee % 2 == 0:
                        if FP8_WO:
                            nc.scalar.mul(out=dst, in_=src, mul=1.0 / S_WO)
                        else:
                            nc.scalar.copy(out=dst, in_=src)
                    else:
                        if FP8_WO:
                            nc.vector.tensor_scalar_mul(out=dst, in0=src,
                                                        scalar1=1.0 / S_WO)
                        else:
                            nc.vector.tensor_copy(out=dst, in_=src)
                nc.sync.dma_start(
                    out=d_out[:, hh * (HIDDEN // 2):(hh + 1) * (HIDDEN // 2)],
                    in_=out_sb)

    return nc


def _build(reps=1):
    key = ("nc", reps, COLLECTIVE, CC_SINGLETON, STAGE, FP8_CT, FP8_WO, FP8_C, FP8_WQB)
    if key not in _BUILD_CACHE:
        from concourse import bacc
        nc = bacc.Bacc("TRN2", target_bir_lowering=False, debug=False,
                       num_devices=N_CORES)
        _emit_kernel(nc, reps=reps, collective=COLLECTIVE)
        nc.compile()
        _BUILD_CACHE[key] = nc
    return _BUILD_CACHE[key]


# --------------------------------------------------------------------------
# entry point
# --------------------------------------------------------------------------

def _run(inputs, **kw):
    from concourse.bass_utils import run_bass_kernel_spmd

    nc = _build()
    shared = _prep_shared(inputs)
    in_maps = [_prep_core(inputs, shared, c) for c in range(N_CORES)]
    res = run_bass_kernel_spmd(nc, in_maps, core_ids=list(range(N_CORES)), **kw)
    out = np.zeros((BSZ, HIDDEN), np.float32)
    for r in res.results:
        out += r["out_partial"]
    return out.reshape(BSZ, 1, HIDDEN), res


def kernel(**inputs):
    out, _ = _run(inputs)
    return out


def run_with_trace(inputs, **kw):
    """test-harness helper: returns (output, BassKernelResults)."""
    return _run(inputs, **kw)
